# revision 7
# baseline (speedup 1.0000x reference)
"""Trainium2 Bass kernel for the GCN graph classifier (2x GCNConv + mean-pool + linear).

Strategy (8 NeuronCores, SPMD):
- Nodes (and their incident in-edges) are sharded contiguously across the 8 cores;
  the small 128x128 weights are replicated.
- GCN layers are linear, so S @ (x @ W) is computed as (S @ x) @ W: propagate raw
  features first (gather + one-hot matmul scatter-add on the PE), then apply W.
  This makes every matmul transpose-free.
- Per 128-edge chunk: dma_gather the 128 source rows, scale by dinv[src] (+ cast to
  bf16) on the scalar engine, build the one-hot selection matrix P[e, n] =
  (dst_local[e] == n) on the vector engine (iota + is_equal), and accumulate
  aggT += Mx^T @ P into PSUM on the tensor engine.
- dinv[dst] scaling and bias/relu are fused into the activation that drains PSUM.
  Biases are added exactly via a rank-1 matmul (outer(sqrt(deg), b)).
- Two launches: layer 1 emits dinv-prescaled activations per shard; the host
  concatenates shards (all-gather) and feeds layer 2, which also does the one-hot
  pooling matmul (per-graph partial sums).
- Host side: index bookkeeping only (degree counts, edge bucketing by dst tile,
  int16 gather index packing) plus the final 8-way partial reduction and the tiny
  [64,128] @ [128,2] classifier.
"""
import sys
from contextlib import ExitStack

import numpy as np
import ml_dtypes

for _p in ("/opt/trn_rl_repo", "/root/.axon_site/_ro/trn_rl_repo"):
    if _p not in sys.path:
        sys.path.append(_p)

import concourse.bass as bass
import concourse.bacc as bacc
import concourse.mybir as mybir
import concourse.tile as tile
from concourse import bass_utils

F32 = mybir.dt.float32
BF16 = mybir.dt.bfloat16
I16 = mybir.dt.int16

# ---- fixed problem geometry (50000 nodes, 800000 edges, 64 graphs, 128 feats)
NC = 8          # cores
NT = 49         # dst tiles of 128 nodes per core
CLO = 10        # chunks (128 edges) per tile with src < SPLIT
CHI = 10        # chunks per tile with src >= SPLIT
GRP = 4         # tiles per gather group
NGRAPH = 64
F = 128
NPAD = NC * NT * 128          # 50176
SPLIT = NPAD // 2             # 25088 (int16 gather index limit)
NCHUNK = CLO + CHI

_GROUPS = []
_t = 0
while _t < NT:
    _n = min(GRP, NT - _t)
    _GROUPS.append((_t, _n))
    _t += _n


def _wrap16(arr_i16):
    """int16 [M*16] -> [128, M]: element i at [i%16, i//16], replicated across the
    8 GPSIMD Q7-core partition groups (HW reads its group's copy)."""
    total = arr_i16.shape[0]
    block = arr_i16.reshape(total // 16, 16).T
    return np.tile(block, (8, 1)).copy()


def _preprocess(x, edge_index, batch):
    N = x.shape[0]
    src = np.concatenate([np.asarray(edge_index[0], dtype=np.int64),
                          np.arange(N, dtype=np.int64)])
    dst = np.concatenate([np.asarray(edge_index[1], dtype=np.int64),
                          np.arange(N, dtype=np.int64)])

    deg = np.bincount(dst, minlength=NPAD).astype(np.float64)
    dinv = np.where(deg > 0, 1.0 / np.sqrt(np.maximum(deg, 1.0)), 0.0).astype(np.float32)
    sqd = np.where(deg > 0, np.sqrt(np.maximum(deg, 1.0)), 0.0).astype(np.float32)

    tile_of = (dst >> 7).astype(np.int64)
    order = np.argsort(tile_of, kind="stable")
    src_s, dst_s = src[order], dst[order]
    tile_s = tile_of[order]
    NTILES = NPAD // 128
    starts = np.searchsorted(tile_s, np.arange(NTILES))
    ends = np.searchsorted(tile_s, np.arange(NTILES), side="right")

    x_pad = np.zeros((NPAD, F), dtype=np.float32)
    x_pad[:N] = np.asarray(x, dtype=np.float32)

    iota128 = np.broadcast_to(np.arange(128, dtype=np.float32), (128, 128)).astype(ml_dtypes.bfloat16)
    iota64 = np.broadcast_to(np.arange(NGRAPH, dtype=np.float32), (128, NGRAPH)).astype(ml_dtypes.bfloat16)

    batch_pad = np.full(NPAD, -1.0, dtype=np.float32)
    batch_pad[:N] = np.asarray(batch, dtype=np.float32)

    in_maps = []
    for c in range(NC):
        ilo = np.zeros((NT, CLO * 128), dtype=np.int16)
        ihi = np.zeros((NT, CHI * 128), dtype=np.int16)
        lcol = np.full((NT, NCHUNK * 128), -1.0, dtype=np.float32)
        dsrc = np.zeros((NT, NCHUNK * 128), dtype=np.float32)
        for t in range(NT):
            gt = c * NT + t
            s, e = starts[gt], ends[gt]
            es, ed = src_s[s:e], dst_s[s:e]
            lo_m = es < SPLIT
            hs, hd = es[lo_m], ed[lo_m]
            n = len(hs)
            assert n <= CLO * 128, f"lo overflow {n}"
            ilo[t, :n] = hs.astype(np.int16)
            lcol[t, :n] = (hd - gt * 128).astype(np.float32)
            dsrc[t, :n] = dinv[hs]
            hs, hd = es[~lo_m], ed[~lo_m]
            n = len(hs)
            assert n <= CHI * 128, f"hi overflow {n}"
            ihi[t, :n] = (hs - SPLIT).astype(np.int16)
            lcol[t, CLO * 128:CLO * 128 + n] = (hd - gt * 128).astype(np.float32)
            dsrc[t, CLO * 128:CLO * 128 + n] = dinv[hs]
        nodes = np.arange(c * NT * 128, (c + 1) * NT * 128)
        in_maps.append({
            "ilo": _wrap16(ilo.reshape(-1)),
            "ihi": _wrap16(ihi.reshape(-1)),
            "lcol": lcol.reshape(NT * NCHUNK, 128).T.copy(),
            "dsrc": dsrc.reshape(NT * NCHUNK, 128).T.copy(),
            "ddst": dinv[nodes].reshape(NT, 128).T.copy(),
            "gcol": batch_pad[nodes].reshape(NT, 128).T.copy(),
            "sqd": sqd[nodes].reshape(1, NT * 128).copy(),
            "io128": np.asarray(iota128), "io64": np.asarray(iota64),
        })
    counts = np.bincount(np.asarray(batch, dtype=np.int64), minlength=NGRAPH).astype(np.float32)
    return x_pad, in_maps, counts


def _emit_layer(tc, outs, ins, li):
    """li=0: gather f32 x -> relu1 (prescaled, bf16) shard out.
    li=1: gather bf16 r1full -> pool partials out."""
    nc = tc.nc
    Relu = mybir.ActivationFunctionType.Relu
    Copy = mybir.ActivationFunctionType.Copy
    ISEQ = mybir.AluOpType.is_equal

    ctx = ExitStack()
    const = ctx.enter_context(tc.tile_pool(name="const", bufs=1))
    glo = ctx.enter_context(tc.tile_pool(name="glo", bufs=2))
    ghi = ctx.enter_context(tc.tile_pool(name="ghi", bufs=2))
    small = ctx.enter_context(tc.tile_pool(name="small", bufs=8))
    work = ctx.enter_context(tc.tile_pool(name="work", bufs=4))
    psA = ctx.enter_context(tc.tile_pool(name="psA", bufs=2, space="PSUM"))
    psB = ctx.enter_context(tc.tile_pool(name="psB", bufs=2, space="PSUM"))
    psP = ctx.enter_context(tc.tile_pool(name="psP", bufs=1, space="PSUM"))

    names = ["ilo", "ihi", "lcol", "ddst", "sqd", "W", "b", "io128"]
    if li == 0:
        names += ["dsrc"]
    else:
        names += ["gcol", "io64"]
    cs = {}
    for k in names:
        ap = ins[k]
        t = const.tile(list(ap.shape), ap.tensor.dtype, tag=k, name=f"c_{k}")
        nc.sync.dma_start(t[:], ap[:])
        cs[k] = t

    if li == 0:
        src_lo, src_hi = ins["x"][:, :], ins["x"][SPLIT:, :]
        gdt = F32
    else:
        src_lo, src_hi = ins["r1"][:, :], ins["r1"][SPLIT:, :]
        gdt = BF16
        poolps = psP.tile([NGRAPH, F], F32, name="poolps")

    for (t0, ntg) in _GROUPS:
        nlo, nhi = ntg * CLO, ntg * CHI
        gl = glo.tile([128, nlo, F], gdt, tag="glo", name="gl")
        gh = ghi.tile([128, nhi, F], gdt, tag="ghi", name="gh")
        nc.gpsimd.dma_gather(
            out_ap=gl[:], in_ap=src_lo,
            idxs_ap=cs["ilo"][:, t0 * CLO * 8:(t0 + ntg) * CLO * 8],
            num_idxs=nlo * 128, num_idxs_reg=nlo * 128, elem_size=F,
            single_packet=False)
        nc.gpsimd.dma_gather(
            out_ap=gh[:], in_ap=src_hi,
            idxs_ap=cs["ihi"][:, t0 * CHI * 8:(t0 + ntg) * CHI * 8],
            num_idxs=nhi * 128, num_idxs_reg=nhi * 128, elem_size=F,
            single_packet=False)

        for ti in range(ntg):
            t = t0 + ti
            agg = psA.tile([128, 128], F32, name="agg")
            for c in range(NCHUNK):
                q = t * NCHUNK + c
                if c < CLO:
                    gsrc = gl[:, ti * CLO + c, :]
                else:
                    gsrc = gh[:, ti * CHI + (c - CLO), :]
                if li == 0:
                    # cast f32 -> bf16 and scale by dinv[src]; alternate the op
                    # between DVE and ACT so neither engine serializes the chain
                    mxt = small.tile([128, F], BF16, tag="mx", name="mx")
                    if c % 2 == 0:
                        nc.vector.tensor_scalar(mxt[:], gsrc, cs["dsrc"][:, q:q + 1],
                                                None, mybir.AluOpType.mult)
                    else:
                        nc.scalar.activation(mxt[:], gsrc, Copy, scale=cs["dsrc"][:, q:q + 1])
                    mxap = mxt[:]
                else:
                    mxap = gsrc
                pt = small.tile([128, 128], BF16, tag="p", name="pt")
                nc.vector.tensor_scalar(pt[:], cs["io128"][:], cs["lcol"][:, q:q + 1], None, ISEQ)
                nc.tensor.matmul(agg[:], lhsT=mxap, rhs=pt[:],
                                 start=(c == 0), stop=(c == NCHUNK - 1))
            aggs = work.tile([128, 128], F32, tag="aggT", name="aggs")
            nc.vector.tensor_copy(aggs[:], agg[:])
            outp = psB.tile([128, 128], F32, name="outp")
            nc.tensor.matmul(outp[:], lhsT=aggs[:], rhs=cs["W"][:], start=True, stop=False)
            nc.tensor.matmul(outp[:], lhsT=cs["sqd"][0:1, t * 128:(t + 1) * 128],
                             rhs=cs["b"][0:1, :], start=False, stop=True)
            if li == 0:
                tmp = work.tile([128, 128], F32, tag="tmp", name="tmp")
                nc.scalar.activation(tmp[:], outp[:], Relu, scale=cs["ddst"][:, t:t + 1])
                r1t = small.tile([128, F], BF16, tag="r1", name="r1t")
                nc.scalar.activation(r1t[:], tmp[:], Copy, scale=cs["ddst"][:, t:t + 1])
                nc.sync.dma_start(outs["r1"][t * 128:(t + 1) * 128, :], r1t[:])
            else:
                r2t = small.tile([128, F], BF16, tag="r2", name="r2t")
                nc.scalar.activation(r2t[:], outp[:], Relu, scale=cs["ddst"][:, t:t + 1])
                bt = small.tile([128, NGRAPH], BF16, tag="bt", name="bt")
                nc.vector.tensor_scalar(bt[:], cs["io64"][:], cs["gcol"][:, t:t + 1], None, ISEQ)
                nc.tensor.matmul(poolps[:], lhsT=bt[:], rhs=r2t[:],
                                 start=(t == 0), stop=(t == NT - 1))

    if li == 1:
        pool_sb = work.tile([NGRAPH, F], F32, tag="pool", name="pool_sb")
        nc.vector.tensor_copy(pool_sb[:], poolps[:])
        nc.sync.dma_start(outs["pool"][:, :], pool_sb[:])
    ctx.close()


_BUILT = {}


def _build(li):
    if li in _BUILT:
        return _BUILT[li]
    nc = bacc.Bacc("TRN2", target_bir_lowering=False, debug=False, num_devices=NC)
    specs = {
        "ilo": ([128, NT * CLO * 8], I16),
        "ihi": ([128, NT * CHI * 8], I16),
        "lcol": ([128, NT * NCHUNK], F32),
        "ddst": ([128, NT], F32),
        "sqd": ([1, NT * 128], F32),
        "W": ([F, F], F32), "b": ([1, F], F32),
        "io128": ([128, 128], BF16),
    }
    if li == 0:
        specs["x"] = ([NPAD, F], F32)
        specs["dsrc"] = ([128, NT * NCHUNK], F32)
    else:
        specs["r1"] = ([NPAD, F], BF16)
        specs["gcol"] = ([128, NT], F32)
        specs["io64"] = ([128, NGRAPH], BF16)
    ins = {k: nc.dram_tensor(k, shp, dt, kind="ExternalInput").ap()
           for k, (shp, dt) in specs.items()}
    if li == 0:
        outs = {"r1": nc.dram_tensor("r1", [NT * 128, F], BF16, kind="ExternalOutput").ap()}
    else:
        outs = {"pool": nc.dram_tensor("pool", [NGRAPH, F], F32, kind="ExternalOutput").ap()}
    with tile.TileContext(nc) as tc:
        _emit_layer(tc, outs, ins, li)
    nc.compile()
    _BUILT[li] = nc
    return nc


def kernel(x, edge_index, batch, W1, b1, W2, b2, Wc, bc, _trace=False):
    x = np.asarray(x)
    x_pad, in_maps, counts = _preprocess(x, edge_index, batch)

    m1 = []
    for m in in_maps:
        m1.append({k: m[k] for k in ["ilo", "ihi", "lcol", "dsrc", "ddst", "sqd", "io128"]}
                  | {"x": x_pad,
                     "W": np.asarray(W1, np.float32),
                     "b": np.asarray(b1, np.float32).reshape(1, F)})
    nc1 = _build(0)
    import time as _time
    _t0 = _time.time()
    res1 = bass_utils.run_bass_kernel_spmd(nc1, m1, core_ids=list(range(NC)), trace=_trace)
    _t1 = _time.time()
    r1_full = np.concatenate([np.asarray(res1.results[c]["r1"]) for c in range(NC)], axis=0)

    m2 = []
    for m in in_maps:
        m2.append({k: m[k] for k in ["ilo", "ihi", "lcol", "ddst", "sqd", "gcol", "io128", "io64"]}
                  | {"r1": r1_full,
                     "W": np.asarray(W2, np.float32),
                     "b": np.asarray(b2, np.float32).reshape(1, F)})
    nc2 = _build(1)
    _t2 = _time.time()
    res2 = bass_utils.run_bass_kernel_spmd(nc2, m2, core_ids=list(range(NC)), trace=_trace)
    _t3 = _time.time()
    kernel._launch_walls = (_t1 - _t0, _t3 - _t2)

    if _trace:
        kernel._last = (res1, res2)
    pooled = np.sum(np.stack([np.asarray(res2.results[c]["pool"], np.float64)
                              for c in range(NC)]), axis=0)
    pooled /= np.maximum(counts, 1.0)[:, None]
    out = pooled @ np.asarray(Wc, np.float64) + np.asarray(bc, np.float64)
    return out.astype(np.float32)


# revision 8
# speedup vs baseline: 1.0109x; 1.0109x over previous
"""Trainium2 Bass kernel for the GCN graph classifier (2x GCNConv + mean-pool + linear).

Strategy (8 NeuronCores, SPMD):
- Nodes (and their incident in-edges) are sharded contiguously across the 8 cores;
  the small 128x128 weights are replicated.
- GCN layers are linear, so S @ (x @ W) is computed as (S @ x) @ W: propagate raw
  features first (gather + one-hot matmul scatter-add on the PE), then apply W.
  This makes every matmul transpose-free.
- Per 128-edge chunk: dma_gather the 128 source rows, scale by dinv[src] (+ cast to
  bf16) on the scalar engine, build the one-hot selection matrix P[e, n] =
  (dst_local[e] == n) on the vector engine (iota + is_equal), and accumulate
  aggT += Mx^T @ P into PSUM on the tensor engine.
- dinv[dst] scaling and bias/relu are fused into the activation that drains PSUM.
  Biases are added exactly via a rank-1 matmul (outer(sqrt(deg), b)).
- Two launches: layer 1 emits dinv-prescaled activations per shard; the host
  concatenates shards (all-gather) and feeds layer 2, which also does the one-hot
  pooling matmul (per-graph partial sums).
- Host side: index bookkeeping only (degree counts, edge bucketing by dst tile,
  int16 gather index packing) plus the final 8-way partial reduction and the tiny
  [64,128] @ [128,2] classifier.
"""
import sys
from contextlib import ExitStack

import numpy as np
import ml_dtypes

for _p in ("/opt/trn_rl_repo", "/root/.axon_site/_ro/trn_rl_repo"):
    if _p not in sys.path:
        sys.path.append(_p)

import concourse.bass as bass
import concourse.bacc as bacc
import concourse.mybir as mybir
import concourse.tile as tile
from concourse import bass_utils

F32 = mybir.dt.float32
BF16 = mybir.dt.bfloat16
I16 = mybir.dt.int16

# ---- fixed problem geometry (50000 nodes, 800000 edges, 64 graphs, 128 feats)
NC = 8          # cores
NT = 49         # dst tiles of 128 nodes per core
CLO = 10        # chunks (128 edges) per tile with src < SPLIT
CHI = 10        # chunks per tile with src >= SPLIT
GRP = 4         # tiles per gather group
NGRAPH = 64
F = 128
NPAD = NC * NT * 128          # 50176
SPLIT = NPAD // 2             # 25088 (int16 gather index limit)
NCHUNK = CLO + CHI

_GROUPS = []
_t = 0
while _t < NT:
    _n = min(GRP, NT - _t)
    _GROUPS.append((_t, _n))
    _t += _n


def _wrap16(arr_i16):
    """int16 [M*16] -> [128, M]: element i at [i%16, i//16], replicated across the
    8 GPSIMD Q7-core partition groups (HW reads its group's copy)."""
    total = arr_i16.shape[0]
    block = arr_i16.reshape(total // 16, 16).T
    return np.tile(block, (8, 1)).copy()


def _preprocess(x, edge_index, batch):
    N = x.shape[0]
    src = np.concatenate([np.asarray(edge_index[0], dtype=np.int64),
                          np.arange(N, dtype=np.int64)])
    dst = np.concatenate([np.asarray(edge_index[1], dtype=np.int64),
                          np.arange(N, dtype=np.int64)])

    deg = np.bincount(dst, minlength=NPAD).astype(np.float64)
    dinv = np.where(deg > 0, 1.0 / np.sqrt(np.maximum(deg, 1.0)), 0.0).astype(np.float32)
    sqd = np.where(deg > 0, np.sqrt(np.maximum(deg, 1.0)), 0.0).astype(np.float32)

    tile_of = (dst >> 7).astype(np.int64)
    order = np.argsort(tile_of, kind="stable")
    src_s, dst_s = src[order], dst[order]
    tile_s = tile_of[order]
    NTILES = NPAD // 128
    starts = np.searchsorted(tile_s, np.arange(NTILES))
    ends = np.searchsorted(tile_s, np.arange(NTILES), side="right")

    x_pad = np.zeros((NPAD, F), dtype=ml_dtypes.bfloat16)
    x_pad[:N] = np.asarray(x, dtype=np.float32).astype(ml_dtypes.bfloat16)

    iota128 = np.broadcast_to(np.arange(128, dtype=np.float32), (128, 128)).astype(ml_dtypes.bfloat16)
    iota64 = np.broadcast_to(np.arange(NGRAPH, dtype=np.float32), (128, NGRAPH)).astype(ml_dtypes.bfloat16)

    batch_pad = np.full(NPAD, -1.0, dtype=np.float32)
    batch_pad[:N] = np.asarray(batch, dtype=np.float32)

    in_maps = []
    for c in range(NC):
        ilo = np.zeros((NT, CLO * 128), dtype=np.int16)
        ihi = np.zeros((NT, CHI * 128), dtype=np.int16)
        lcol = np.full((NT, NCHUNK * 128), -1.0, dtype=np.float32)
        dsrc = np.zeros((NT, NCHUNK * 128), dtype=np.float32)
        for t in range(NT):
            gt = c * NT + t
            s, e = starts[gt], ends[gt]
            es, ed = src_s[s:e], dst_s[s:e]
            lo_m = es < SPLIT
            hs, hd = es[lo_m], ed[lo_m]
            n = len(hs)
            assert n <= CLO * 128, f"lo overflow {n}"
            ilo[t, :n] = hs.astype(np.int16)
            lcol[t, :n] = (hd - gt * 128).astype(np.float32)
            dsrc[t, :n] = dinv[hs]
            hs, hd = es[~lo_m], ed[~lo_m]
            n = len(hs)
            assert n <= CHI * 128, f"hi overflow {n}"
            ihi[t, :n] = (hs - SPLIT).astype(np.int16)
            lcol[t, CLO * 128:CLO * 128 + n] = (hd - gt * 128).astype(np.float32)
            dsrc[t, CLO * 128:CLO * 128 + n] = dinv[hs]
        nodes = np.arange(c * NT * 128, (c + 1) * NT * 128)
        in_maps.append({
            "ilo": _wrap16(ilo.reshape(-1)),
            "ihi": _wrap16(ihi.reshape(-1)),
            "lcol": lcol.reshape(NT * NCHUNK, 128).T.copy(),
            "dsrc": dsrc.reshape(NT * NCHUNK, 128).T.copy(),
            "ddst": dinv[nodes].reshape(NT, 128).T.copy(),
            "gcol": batch_pad[nodes].reshape(NT, 128).T.copy(),
            "sqd": sqd[nodes].reshape(1, NT * 128).copy(),
            "io128": np.asarray(iota128), "io64": np.asarray(iota64),
        })
    counts = np.bincount(np.asarray(batch, dtype=np.int64), minlength=NGRAPH).astype(np.float32)
    return x_pad, in_maps, counts


def _emit_layer(tc, outs, ins, li):
    """li=0: gather f32 x -> relu1 (prescaled, bf16) shard out.
    li=1: gather bf16 r1full -> pool partials out."""
    nc = tc.nc
    Relu = mybir.ActivationFunctionType.Relu
    Copy = mybir.ActivationFunctionType.Copy
    ISEQ = mybir.AluOpType.is_equal

    ctx = ExitStack()
    const = ctx.enter_context(tc.tile_pool(name="const", bufs=1))
    glo = ctx.enter_context(tc.tile_pool(name="glo", bufs=2))
    ghi = ctx.enter_context(tc.tile_pool(name="ghi", bufs=2))
    small = ctx.enter_context(tc.tile_pool(name="small", bufs=8))
    work = ctx.enter_context(tc.tile_pool(name="work", bufs=4))
    psA = ctx.enter_context(tc.tile_pool(name="psA", bufs=2, space="PSUM"))
    psB = ctx.enter_context(tc.tile_pool(name="psB", bufs=2, space="PSUM"))
    psP = ctx.enter_context(tc.tile_pool(name="psP", bufs=1, space="PSUM"))

    names = ["ilo", "ihi", "lcol", "ddst", "sqd", "W", "b", "io128"]
    if li == 0:
        names += ["dsrc"]
    else:
        names += ["gcol", "io64"]
    cs = {}
    for k in names:
        ap = ins[k]
        t = const.tile(list(ap.shape), ap.tensor.dtype, tag=k, name=f"c_{k}")
        nc.sync.dma_start(t[:], ap[:])
        cs[k] = t

    if li == 0:
        src_lo, src_hi = ins["x"][:, :], ins["x"][SPLIT:, :]
        gdt = BF16
    else:
        src_lo, src_hi = ins["r1"][:, :], ins["r1"][SPLIT:, :]
        gdt = BF16
        poolps = psP.tile([NGRAPH, F], F32, name="poolps")

    for (t0, ntg) in _GROUPS:
        nlo, nhi = ntg * CLO, ntg * CHI
        gl = glo.tile([128, nlo, F], gdt, tag="glo", name="gl")
        gh = ghi.tile([128, nhi, F], gdt, tag="ghi", name="gh")
        nc.gpsimd.dma_gather(
            out_ap=gl[:], in_ap=src_lo,
            idxs_ap=cs["ilo"][:, t0 * CLO * 8:(t0 + ntg) * CLO * 8],
            num_idxs=nlo * 128, num_idxs_reg=nlo * 128, elem_size=F,
            single_packet=False)
        nc.gpsimd.dma_gather(
            out_ap=gh[:], in_ap=src_hi,
            idxs_ap=cs["ihi"][:, t0 * CHI * 8:(t0 + ntg) * CHI * 8],
            num_idxs=nhi * 128, num_idxs_reg=nhi * 128, elem_size=F,
            single_packet=False)

        for ti in range(ntg):
            t = t0 + ti
            agg = psA.tile([128, 128], F32, name="agg")
            for c in range(NCHUNK):
                q = t * NCHUNK + c
                if c < CLO:
                    gsrc = gl[:, ti * CLO + c, :]
                else:
                    gsrc = gh[:, ti * CHI + (c - CLO), :]
                pt = small.tile([128, 128], BF16, tag="p", name="pt")
                if li == 0:
                    # P[e, n] = (dst_local[e] == n) * dinv[src_e] — dinv[src]
                    # rides the fused second ALU op for free
                    nc.vector.tensor_scalar(pt[:], cs["io128"][:], cs["lcol"][:, q:q + 1],
                                            cs["dsrc"][:, q:q + 1], ISEQ,
                                            mybir.AluOpType.mult)
                else:
                    nc.vector.tensor_scalar(pt[:], cs["io128"][:], cs["lcol"][:, q:q + 1], None, ISEQ)
                nc.tensor.matmul(agg[:], lhsT=gsrc, rhs=pt[:],
                                 start=(c == 0), stop=(c == NCHUNK - 1))
            aggs = work.tile([128, 128], F32, tag="aggT", name="aggs")
            nc.vector.tensor_copy(aggs[:], agg[:])
            outp = psB.tile([128, 128], F32, name="outp")
            nc.tensor.matmul(outp[:], lhsT=aggs[:], rhs=cs["W"][:], start=True, stop=False)
            nc.tensor.matmul(outp[:], lhsT=cs["sqd"][0:1, t * 128:(t + 1) * 128],
                             rhs=cs["b"][0:1, :], start=False, stop=True)
            if li == 0:
                tmp = work.tile([128, 128], F32, tag="tmp", name="tmp")
                nc.scalar.activation(tmp[:], outp[:], Relu, scale=cs["ddst"][:, t:t + 1])
                r1t = small.tile([128, F], BF16, tag="r1", name="r1t")
                nc.scalar.activation(r1t[:], tmp[:], Copy, scale=cs["ddst"][:, t:t + 1])
                nc.sync.dma_start(outs["r1"][t * 128:(t + 1) * 128, :], r1t[:])
            else:
                r2t = small.tile([128, F], BF16, tag="r2", name="r2t")
                nc.scalar.activation(r2t[:], outp[:], Relu, scale=cs["ddst"][:, t:t + 1])
                bt = small.tile([128, NGRAPH], BF16, tag="bt", name="bt")
                nc.vector.tensor_scalar(bt[:], cs["io64"][:], cs["gcol"][:, t:t + 1], None, ISEQ)
                nc.tensor.matmul(poolps[:], lhsT=bt[:], rhs=r2t[:],
                                 start=(t == 0), stop=(t == NT - 1))

    if li == 1:
        pool_sb = work.tile([NGRAPH, F], F32, tag="pool", name="pool_sb")
        nc.vector.tensor_copy(pool_sb[:], poolps[:])
        nc.sync.dma_start(outs["pool"][:, :], pool_sb[:])
    ctx.close()


_BUILT = {}


def _build(li):
    if li in _BUILT:
        return _BUILT[li]
    nc = bacc.Bacc("TRN2", target_bir_lowering=False, debug=False, num_devices=NC)
    specs = {
        "ilo": ([128, NT * CLO * 8], I16),
        "ihi": ([128, NT * CHI * 8], I16),
        "lcol": ([128, NT * NCHUNK], F32),
        "ddst": ([128, NT], F32),
        "sqd": ([1, NT * 128], F32),
        "W": ([F, F], F32), "b": ([1, F], F32),
        "io128": ([128, 128], BF16),
    }
    if li == 0:
        specs["x"] = ([NPAD, F], BF16)
        specs["dsrc"] = ([128, NT * NCHUNK], F32)
    else:
        specs["r1"] = ([NPAD, F], BF16)
        specs["gcol"] = ([128, NT], F32)
        specs["io64"] = ([128, NGRAPH], BF16)
    ins = {k: nc.dram_tensor(k, shp, dt, kind="ExternalInput").ap()
           for k, (shp, dt) in specs.items()}
    if li == 0:
        outs = {"r1": nc.dram_tensor("r1", [NT * 128, F], BF16, kind="ExternalOutput").ap()}
    else:
        outs = {"pool": nc.dram_tensor("pool", [NGRAPH, F], F32, kind="ExternalOutput").ap()}
    with tile.TileContext(nc) as tc:
        _emit_layer(tc, outs, ins, li)
    nc.compile()
    _BUILT[li] = nc
    return nc


def kernel(x, edge_index, batch, W1, b1, W2, b2, Wc, bc, _trace=False):
    x = np.asarray(x)
    x_pad, in_maps, counts = _preprocess(x, edge_index, batch)

    m1 = []
    for m in in_maps:
        m1.append({k: m[k] for k in ["ilo", "ihi", "lcol", "dsrc", "ddst", "sqd", "io128"]}
                  | {"x": x_pad,
                     "W": np.asarray(W1, np.float32),
                     "b": np.asarray(b1, np.float32).reshape(1, F)})
    nc1 = _build(0)
    import time as _time
    _t0 = _time.time()
    res1 = bass_utils.run_bass_kernel_spmd(nc1, m1, core_ids=list(range(NC)), trace=_trace)
    _t1 = _time.time()
    r1_full = np.concatenate([np.asarray(res1.results[c]["r1"]) for c in range(NC)], axis=0)

    m2 = []
    for m in in_maps:
        m2.append({k: m[k] for k in ["ilo", "ihi", "lcol", "ddst", "sqd", "gcol", "io128", "io64"]}
                  | {"r1": r1_full,
                     "W": np.asarray(W2, np.float32),
                     "b": np.asarray(b2, np.float32).reshape(1, F)})
    nc2 = _build(1)
    _t2 = _time.time()
    res2 = bass_utils.run_bass_kernel_spmd(nc2, m2, core_ids=list(range(NC)), trace=_trace)
    _t3 = _time.time()
    kernel._launch_walls = (_t1 - _t0, _t3 - _t2)

    if _trace:
        kernel._last = (res1, res2)
    pooled = np.sum(np.stack([np.asarray(res2.results[c]["pool"], np.float64)
                              for c in range(NC)]), axis=0)
    pooled /= np.maximum(counts, 1.0)[:, None]
    out = pooled @ np.asarray(Wc, np.float64) + np.asarray(bc, np.float64)
    return out.astype(np.float32)
